# revision 3
# baseline (speedup 1.0000x reference)
"""Trainium2 Bass kernel for nn_CartesianToJacobi.

Computes, per batch row b (N=16 bodies, D=3 dims):
    A = jacobi_matrix(m[b]);  qj[b] = A @ q[b];  vj[b] = A @ v[b]

The matrix product collapses to weighted prefix sums.  With
M_i = cumsum(m)_i the output rows obey (a_i = M_{i-1}/M_i, a_0 = 0):
    w_1 = x_1 - x_0;   w_{t+1} = a_t w_t + (x_{t+1} - x_t)
    out_i = w_i (i>=1),  out_0 = c_{N-1} = x_{N-1} - a_{N-1} w_{N-1}
so one full-width diff + one DVE scan per d produce everything; a_0 = 0
makes the recurrence self-reset at row starts, letting one scan chain
across batch rows and across the fused q|v halves of a plane.

Performance notes (measured on these cores, slope-timed):
- All HBM traffic is fp16 (tolerance 2e-2, measured error ~1e-3);
  DMA is nowhere near the bottleneck at the observed bandwidth.
- The DVE scan is the bottleneck; it runs ~1 slot/cycle ONLY for
  contiguous, 32-byte-aligned operands.  Strided or odd-aligned fp16
  writes are ~3x slower (read-modify-write).  Hence the d-major layout:

d-major: q/v/qj/vj DRAM tensors are [D, BS, N] (host transposes),
so every scan runs over contiguous slots.  Scan outputs are shifted by
S=16 slots into a padded plane to keep writes 32-byte aligned; the diff is
full-width (slot N-1 reads the next row's x_0, which cancels out of the
row-0 fixup algebraically).  On-chip layout per chunk: (d, x, c, n).
"""

import numpy as np

import concourse.bacc as bacc
import concourse.mybir as mybir
import concourse.tile as tile
from concourse.bass_utils import run_bass_kernel_spmd

B, N, D = 131072, 16, 3
NCORES = 8
P = 128
S = 16  # scan output shift (alignment pad)


def build_nc(BS=B // NCORES, CC=32, bufs=3, reps=1, amult16=False):
    C = BS // P
    assert C % CC == 0
    nchunks = C // CC
    CN = CC * N
    E = 2 * CN  # slots per d-plane (x in {q,v} fused)
    OXP = E + S  # ox plane length
    f16 = mybir.dt.float16
    f32 = mybir.dt.float32
    Alu = mybir.AluOpType

    nc = bacc.Bacc("TRN2", num_devices=NCORES)
    m_d = nc.dram_tensor("m", [BS, N], f16, kind="ExternalInput")
    q_d = nc.dram_tensor("q", [D, BS, N], f16, kind="ExternalInput")
    v_d = nc.dram_tensor("v", [D, BS, N], f16, kind="ExternalInput")
    qj_d = nc.dram_tensor("qj", [D, BS, N], f16, kind="ExternalOutput")
    vj_d = nc.dram_tensor("vj", [D, BS, N], f16, kind="ExternalOutput")

    mv = m_d.ap().rearrange("(p c) n -> p c n", p=P)
    qv = q_d.ap().rearrange("d (p c) n -> p d c n", p=P)
    vv = v_d.ap().rearrange("d (p c) n -> p d c n", p=P)
    qjv = qj_d.ap().rearrange("d (p c) n -> p d c n", p=P)
    vjv = vj_d.ap().rearrange("d (p c) n -> p d c n", p=P)

    with tile.TileContext(nc) as tc:
        with (
            tc.tile_pool(name="const", bufs=1) as cpool,
            tc.tile_pool(name="work", bufs=bufs) as pool,
        ):
            g = cpool.tile([P, CN], f16)
            nc.vector.memset(g[:, :], 1.0)
            nc.vector.memset(
                g.rearrange("p (c n) -> p c n", n=N)[:, :, 0:1], 0.0
            )
            # persistent rotating a tiles; slot n=0 pre-zeroed (a_0 = 0)
            ats = []
            for i in range(3):
                at = cpool.tile([P, CN], f16, name=f"a{i}")
                nc.vector.memset(
                    at.rearrange("p (c n) -> p c n", n=N)[:, :, 0:1], 0.0
                )
                ats.append(at)

            for r in range(reps):
                for k in range(nchunks):
                    sl = slice(k * CC, (k + 1) * CC)

                    mt = pool.tile([P, CN], f16, tag="mt")
                    nc.sync.dma_start(
                        out=mt.rearrange("p (c n) -> p c n", n=N),
                        in_=mv[:, sl],
                    )
                    Mt = pool.tile([P, CN], f32, tag="Mt")
                    nc.vector.tensor_tensor_scan(
                        Mt[:, :], g[:, :], mt[:, :], 0.0, Alu.mult, Alu.add
                    )
                    rM = pool.tile([P, CN], f32, tag="rM")
                    nc.vector.reciprocal_approx_fast(rM[:, :], Mt[:, :])

                    a = ats[k % len(ats)]
                    a3 = a.rearrange("p (c n) -> p c n", n=N)
                    if amult16:
                        Mt16 = pool.tile([P, CN], f16, tag="Mt16")
                        nc.scalar.copy(Mt16[:, :], Mt[:, :])
                        rM16 = pool.tile([P, CN], f16, tag="rM16")
                        nc.scalar.copy(rM16[:, :], rM[:, :])
                        Mt163 = Mt16.rearrange("p (c n) -> p c n", n=N)
                        rM163 = rM16.rearrange("p (c n) -> p c n", n=N)
                        nc.vector.tensor_mul(
                            a3[:, :, 1:],
                            Mt163[:, :, 0 : N - 1],
                            rM163[:, :, 1:],
                        )
                    else:
                        Mt3 = Mt.rearrange("p (c n) -> p c n", n=N)
                        rM3 = rM.rearrange("p (c n) -> p c n", n=N)
                        nc.vector.tensor_mul(
                            a3[:, :, 1:], Mt3[:, :, 0 : N - 1], rM3[:, :, 1:]
                        )
                    a2 = pool.tile([P, E], f16, tag="a2")
                    nc.scalar.copy(
                        a2.rearrange("p (x cn) -> p x cn", x=2),
                        a[:, :].unsqueeze(1).broadcast_to([P, 2, CN]),
                    )

                    # fused tile, layout (d, x, c, n), +N pad elements so
                    # the full-width diff and X1 staging stay in bounds
                    xt = pool.tile([P, D * E + N], f16, tag="xt")
                    xt5 = xt[:, 0 : D * E].rearrange(
                        "p (d x c n) -> p d x c n", d=D, x=2, n=N
                    )
                    nc.sync.dma_start(out=xt5[:, :, 0], in_=qv[:, :, sl])
                    nc.sync.dma_start(out=xt5[:, :, 1], in_=vv[:, :, sl])
                    nc.scalar.memzero(xt[:, D * E : D * E + N])

                    # X1[e] = xt[e*N + N] (next row's first elem; any value
                    # works algebraically for the row-0 fixup — it cancels)
                    ne = D * 2 * CC
                    X1 = pool.tile([P, ne], f16, tag="X1")
                    nc.scalar.copy(
                        X1.rearrange("p (e o) -> p e o", o=1),
                        xt[:, N : D * E + N].rearrange(
                            "p (e n) -> p e n", n=N
                        )[:, :, 0:1],
                    )

                    # full-width contiguous diff (2x, aligned)
                    dx = pool.tile([P, D * E], f16, tag="dx")
                    nc.vector.tensor_sub(
                        dx[:, :], xt[:, 1 : D * E + 1], xt[:, 0 : D * E]
                    )

                    # per-d contiguous scans, output shifted by S slots
                    ox = pool.tile([P, OXP * D], f16)
                    for d in range(D):
                        nc.vector.tensor_tensor_scan(
                            ox[:, d * OXP + S : d * OXP + S + E],
                            a2[:, :],
                            dx[:, d * E : (d + 1) * E],
                            0.0,
                            Alu.mult,
                            Alu.add,
                        )
                    # overflow of row e sits at store-slot 0 of row e+1 =
                    # plane position (e+1)*N + S-1; last row's lands in the
                    # pad tail... (E-1)+S ... position E+S-1 < OXP.  Fixup:
                    # out[e,0] = c15[e] = X1[e] - overflow[e].
                    oxp = ox.rearrange("p (d s) -> p d s", d=D)
                    s_view = oxp[:, :, N + S - 1 :: N][:, :, 0 : 2 * CC]
                    r0 = pool.tile([P, ne], f16)
                    r03 = r0.rearrange("p (d e) -> p d e", d=D)
                    nc.gpsimd.tensor_sub(
                        r03, X1.rearrange("p (d e) -> p d e", d=D), s_view
                    )
                    nc.scalar.copy(oxp[:, :, S - 1 :: N][:, :, 0 : 2 * CC], r03)

                    # store block of plane d = positions [S-1, S-1+E)
                    qj_src = oxp[:, :, S - 1 : S - 1 + CN].rearrange(
                        "p d (c n) -> p d c n", n=N
                    )
                    vj_src = oxp[:, :, S - 1 + CN : S - 1 + E].rearrange(
                        "p d (c n) -> p d c n", n=N
                    )
                    nc.sync.dma_start(out=qjv[:, :, sl], in_=qj_src)
                    nc.sync.dma_start(out=vjv[:, :, sl], in_=vj_src)

    nc.compile()
    return nc


_CACHE = {}


def _get_nc():
    if "nc" not in _CACHE:
        _CACHE["nc"] = build_nc()
    return _CACHE["nc"]


def prep_core_inputs(m, q, v, i):
    BS = B // NCORES
    sl = slice(i * BS, (i + 1) * BS)
    return {
        "m": np.ascontiguousarray(m[sl]).astype(np.float16),
        "q": np.ascontiguousarray(
            np.asarray(q[sl]).astype(np.float16).transpose(2, 0, 1)
        ),
        "v": np.ascontiguousarray(
            np.asarray(v[sl]).astype(np.float16).transpose(2, 0, 1)
        ),
    }


def kernel(m, q, v):
    import os

    os.environ["BASS_NEVER_TRACE"] = "1"
    nc = _get_nc()
    m = np.asarray(m)
    q = np.asarray(q)
    v = np.asarray(v)
    in_maps = [prep_core_inputs(m, q, v, i) for i in range(NCORES)]
    res = run_bass_kernel_spmd(nc, in_maps, list(range(NCORES))).results
    qj = np.concatenate(
        [res[i]["qj"].transpose(1, 2, 0) for i in range(NCORES)], axis=0
    ).astype(np.float32)
    vj = np.concatenate(
        [res[i]["vj"].transpose(1, 2, 0) for i in range(NCORES)], axis=0
    ).astype(np.float32)
    return qj, vj


if __name__ == "__main__":
    import time
    import jax

    with jax.default_device(jax.devices("cpu")[0]):
        import reference

        inputs = {k: np.asarray(x) for k, x in reference.setup_inputs().items()}
        qj_ref, vj_ref = reference.reference(**inputs)
        qj_ref = np.asarray(qj_ref)
        vj_ref = np.asarray(vj_ref)
    t0 = time.time()
    qj, vj = kernel(**inputs)
    print(f"kernel done in {time.time() - t0:.1f}s")
    for name, got, want in (("qj", qj, qj_ref), ("vj", vj, vj_ref)):
        err = np.abs(got - want).max() / np.abs(want).max()
        print(f"{name}: scale-rel absmax err = {err:.3e}")


# revision 5
# speedup vs baseline: 1.0916x; 1.0916x over previous
"""Trainium2 Bass kernel for nn_CartesianToJacobi.

Computes, per batch row b (N=16 bodies, D=3 dims):
    A = jacobi_matrix(m[b]);  qj[b] = A @ q[b];  vj[b] = A @ v[b]

The matrix product collapses to weighted prefix sums.  With
M_i = cumsum(m)_i the output rows obey (a_i = M_{i-1}/M_i, a_0 = 0):
    w_1 = x_1 - x_0;   w_{t+1} = a_t w_t + (x_{t+1} - x_t)
    out_i = w_i (i>=1),  out_0 = c_{N-1} = x_{N-1} - a_{N-1} w_{N-1}
so one full-width diff + one DVE scan per d produce everything; a_0 = 0
makes the recurrence self-reset at row starts, letting one scan chain
across batch rows and across the fused q|v halves of a plane.

Performance notes (measured on these cores, slope-timed):
- All HBM traffic is fp16 (tolerance 2e-2, measured error ~1e-3);
  DMA is nowhere near the bottleneck at the observed bandwidth.
- The DVE scan is the bottleneck; it runs ~1 slot/cycle ONLY for
  contiguous, 32-byte-aligned operands.  Strided or odd-aligned fp16
  writes are ~3x slower (read-modify-write).  Hence the d-major layout.
- Sustained throughput (many in-NEFF reps) is lower than burst; a
  smaller instruction footprint helps, hence CC=64 (2 chunks/rep).

d-major: q/v/qj/vj DRAM tensors are [D, BS, N] (host transposes),
so every scan runs over contiguous slots.  Scan outputs are shifted by
S=16 slots into a padded plane to keep writes 32-byte aligned; the diff is
full-width (slot N-1 reads the next row's x_0, which cancels out of the
row-0 fixup algebraically).  On-chip layout per chunk: (d, x, c, n).
"""

import numpy as np

import concourse.bacc as bacc
import concourse.mybir as mybir
import concourse.tile as tile
from concourse.bass_utils import run_bass_kernel_spmd

B, N, D = 131072, 16, 3
NCORES = 8
P = 128
S = 16  # scan output shift (alignment pad)


def build_nc(BS=B // NCORES, CC=64, bufs=3, reps=1, amult16=False):
    C = BS // P
    assert C % CC == 0
    nchunks = C // CC
    CN = CC * N
    E = 2 * CN  # slots per d-plane (x in {q,v} fused)
    OXP = E + S  # ox plane length
    f16 = mybir.dt.float16
    f32 = mybir.dt.float32
    Alu = mybir.AluOpType

    nc = bacc.Bacc("TRN2", num_devices=NCORES)
    m_d = nc.dram_tensor("m", [BS, N], f16, kind="ExternalInput")
    q_d = nc.dram_tensor("q", [D, BS, N], f16, kind="ExternalInput")
    v_d = nc.dram_tensor("v", [D, BS, N], f16, kind="ExternalInput")
    qj_d = nc.dram_tensor("qj", [D, BS, N], f16, kind="ExternalOutput")
    vj_d = nc.dram_tensor("vj", [D, BS, N], f16, kind="ExternalOutput")

    mv = m_d.ap().rearrange("(p c) n -> p c n", p=P)
    qv = q_d.ap().rearrange("d (p c) n -> p d c n", p=P)
    vv = v_d.ap().rearrange("d (p c) n -> p d c n", p=P)
    qjv = qj_d.ap().rearrange("d (p c) n -> p d c n", p=P)
    vjv = vj_d.ap().rearrange("d (p c) n -> p d c n", p=P)

    with tile.TileContext(nc) as tc:
        with (
            tc.tile_pool(name="const", bufs=1) as cpool,
            tc.tile_pool(name="work", bufs=bufs) as pool,
        ):
            g = cpool.tile([P, CN], f16)
            nc.vector.memset(g[:, :], 1.0)
            nc.vector.memset(
                g.rearrange("p (c n) -> p c n", n=N)[:, :, 0:1], 0.0
            )
            # persistent rotating a tiles; slot n=0 pre-zeroed (a_0 = 0)
            ats = []
            for i in range(3):
                at = cpool.tile([P, CN], f16, name=f"a{i}")
                nc.vector.memset(
                    at.rearrange("p (c n) -> p c n", n=N)[:, :, 0:1], 0.0
                )
                ats.append(at)

            for r in range(reps):
                for k in range(nchunks):
                    sl = slice(k * CC, (k + 1) * CC)

                    mt = pool.tile([P, CN], f16, tag="mt")
                    nc.sync.dma_start(
                        out=mt.rearrange("p (c n) -> p c n", n=N),
                        in_=mv[:, sl],
                    )
                    Mt = pool.tile([P, CN], f32, tag="Mt")
                    nc.vector.tensor_tensor_scan(
                        Mt[:, :], g[:, :], mt[:, :], 0.0, Alu.mult, Alu.add
                    )
                    rM = pool.tile([P, CN], f32, tag="rM")
                    nc.vector.reciprocal_approx_fast(rM[:, :], Mt[:, :])

                    a = ats[k % len(ats)]
                    a3 = a.rearrange("p (c n) -> p c n", n=N)
                    if amult16:
                        Mt16 = pool.tile([P, CN], f16, tag="Mt16")
                        nc.scalar.copy(Mt16[:, :], Mt[:, :])
                        rM16 = pool.tile([P, CN], f16, tag="rM16")
                        nc.scalar.copy(rM16[:, :], rM[:, :])
                        Mt163 = Mt16.rearrange("p (c n) -> p c n", n=N)
                        rM163 = rM16.rearrange("p (c n) -> p c n", n=N)
                        nc.vector.tensor_mul(
                            a3[:, :, 1:],
                            Mt163[:, :, 0 : N - 1],
                            rM163[:, :, 1:],
                        )
                    else:
                        Mt3 = Mt.rearrange("p (c n) -> p c n", n=N)
                        rM3 = rM.rearrange("p (c n) -> p c n", n=N)
                        nc.vector.tensor_mul(
                            a3[:, :, 1:], Mt3[:, :, 0 : N - 1], rM3[:, :, 1:]
                        )
                    a2 = pool.tile([P, E], f16, tag="a2")
                    nc.scalar.copy(
                        a2.rearrange("p (x cn) -> p x cn", x=2),
                        a[:, :].unsqueeze(1).broadcast_to([P, 2, CN]),
                    )

                    # fused tile, layout (d, x, c, n), +N pad elements so
                    # the full-width diff and X1 staging stay in bounds
                    xt = pool.tile([P, D * E + N], f16, tag="xt")
                    xt5 = xt[:, 0 : D * E].rearrange(
                        "p (d x c n) -> p d x c n", d=D, x=2, n=N
                    )
                    nc.sync.dma_start(out=xt5[:, :, 0], in_=qv[:, :, sl])
                    nc.sync.dma_start(out=xt5[:, :, 1], in_=vv[:, :, sl])
                    nc.scalar.memzero(xt[:, D * E : D * E + N])

                    # X1[e] = xt[e*N + N] (next row's first elem; any value
                    # works algebraically for the row-0 fixup — it cancels)
                    ne = D * 2 * CC
                    X1 = pool.tile([P, ne], f16, tag="X1")
                    nc.scalar.copy(
                        X1.rearrange("p (e o) -> p e o", o=1),
                        xt[:, N : D * E + N].rearrange(
                            "p (e n) -> p e n", n=N
                        )[:, :, 0:1],
                    )

                    # full-width contiguous diff (2x, aligned)
                    dx = pool.tile([P, D * E], f16, tag="dx")
                    nc.vector.tensor_sub(
                        dx[:, :], xt[:, 1 : D * E + 1], xt[:, 0 : D * E]
                    )

                    # per-d contiguous scans, output shifted by S slots
                    ox = pool.tile([P, OXP * D], f16)
                    for d in range(D):
                        nc.vector.tensor_tensor_scan(
                            ox[:, d * OXP + S : d * OXP + S + E],
                            a2[:, :],
                            dx[:, d * E : (d + 1) * E],
                            0.0,
                            Alu.mult,
                            Alu.add,
                        )
                    # overflow of row e sits at store-slot 0 of row e+1 =
                    # plane position (e+1)*N + S-1; last row's lands in the
                    # pad tail... (E-1)+S ... position E+S-1 < OXP.  Fixup:
                    # out[e,0] = c15[e] = X1[e] - overflow[e].
                    oxp = ox.rearrange("p (d s) -> p d s", d=D)
                    s_view = oxp[:, :, N + S - 1 :: N][:, :, 0 : 2 * CC]
                    r0 = pool.tile([P, ne], f16)
                    r03 = r0.rearrange("p (d e) -> p d e", d=D)
                    nc.gpsimd.tensor_sub(
                        r03, X1.rearrange("p (d e) -> p d e", d=D), s_view
                    )
                    nc.scalar.copy(oxp[:, :, S - 1 :: N][:, :, 0 : 2 * CC], r03)

                    # store block of plane d = positions [S-1, S-1+E)
                    qj_src = oxp[:, :, S - 1 : S - 1 + CN].rearrange(
                        "p d (c n) -> p d c n", n=N
                    )
                    vj_src = oxp[:, :, S - 1 + CN : S - 1 + E].rearrange(
                        "p d (c n) -> p d c n", n=N
                    )
                    nc.sync.dma_start(out=qjv[:, :, sl], in_=qj_src)
                    nc.sync.dma_start(out=vjv[:, :, sl], in_=vj_src)

    nc.compile()
    return nc


_CACHE = {}


def _get_nc():
    if "nc" not in _CACHE:
        _CACHE["nc"] = build_nc()
    return _CACHE["nc"]


def prep_core_inputs(m, q, v, i):
    BS = B // NCORES
    sl = slice(i * BS, (i + 1) * BS)
    return {
        "m": np.ascontiguousarray(m[sl]).astype(np.float16),
        "q": np.ascontiguousarray(
            np.asarray(q[sl]).astype(np.float16).transpose(2, 0, 1)
        ),
        "v": np.ascontiguousarray(
            np.asarray(v[sl]).astype(np.float16).transpose(2, 0, 1)
        ),
    }


def kernel(m, q, v):
    import os

    os.environ["BASS_NEVER_TRACE"] = "1"
    nc = _get_nc()
    m = np.asarray(m)
    q = np.asarray(q)
    v = np.asarray(v)
    in_maps = [prep_core_inputs(m, q, v, i) for i in range(NCORES)]
    res = run_bass_kernel_spmd(nc, in_maps, list(range(NCORES))).results
    qj = np.concatenate(
        [res[i]["qj"].transpose(1, 2, 0) for i in range(NCORES)], axis=0
    ).astype(np.float32)
    vj = np.concatenate(
        [res[i]["vj"].transpose(1, 2, 0) for i in range(NCORES)], axis=0
    ).astype(np.float32)
    return qj, vj


if __name__ == "__main__":
    import time
    import jax

    with jax.default_device(jax.devices("cpu")[0]):
        import reference

        inputs = {k: np.asarray(x) for k, x in reference.setup_inputs().items()}
        qj_ref, vj_ref = reference.reference(**inputs)
        qj_ref = np.asarray(qj_ref)
        vj_ref = np.asarray(vj_ref)
    t0 = time.time()
    qj, vj = kernel(**inputs)
    print(f"kernel done in {time.time() - t0:.1f}s")
    for name, got, want in (("qj", qj, qj_ref), ("vj", vj, vj_ref)):
        err = np.abs(got - want).max() / np.abs(want).max()
        print(f"{name}: scale-rel absmax err = {err:.3e}")


# revision 6
# speedup vs baseline: 1.0985x; 1.0064x over previous
"""Trainium2 Bass kernel for nn_CartesianToJacobi.

Computes, per batch row b (N=16 bodies, D=3 dims):
    A = jacobi_matrix(m[b]);  qj[b] = A @ q[b];  vj[b] = A @ v[b]

The matrix product collapses to weighted prefix sums.  With
M_i = cumsum(m)_i the output rows obey (a_i = M_{i-1}/M_i, a_0 = 0):
    w_1 = x_1 - x_0;   w_{t+1} = a_t w_t + (x_{t+1} - x_t)
    out_i = w_i (i>=1),  out_0 = c_{N-1} = x_{N-1} - a_{N-1} w_{N-1}
so one full-width diff + one DVE scan per d produce everything; a_0 = 0
makes the recurrence self-reset at row starts, letting one scan chain
across batch rows and across the fused q|v halves of a plane.

Performance notes (measured on these cores, slope-timed):
- All HBM traffic is fp16 (tolerance 2e-2, measured error ~1e-3);
  DMA is nowhere near the bottleneck at the observed bandwidth.
- The DVE scan is the bottleneck; it runs ~1 slot/cycle ONLY for
  contiguous, 32-byte-aligned operands.  Strided or odd-aligned fp16
  writes are ~3x slower (read-modify-write).  Hence the d-major layout.
- Sustained throughput (many in-NEFF reps) is lower than burst; a
  smaller instruction footprint helps, hence CC=64 (2 chunks/rep).

d-major: q/v/qj/vj DRAM tensors are [D, BS, N] (host transposes),
so every scan runs over contiguous slots.  Scan outputs are shifted by
S=16 slots into a padded plane to keep writes 32-byte aligned; the diff is
full-width (slot N-1 reads the next row's x_0, which cancels out of the
row-0 fixup algebraically).  On-chip layout per chunk: (d, x, c, n).
"""

import numpy as np

import concourse.bacc as bacc
import concourse.mybir as mybir
import concourse.tile as tile
from concourse.bass_utils import run_bass_kernel_spmd

B, N, D = 131072, 16, 3
NCORES = 8
P = 128
S = 16  # scan output shift (alignment pad)


def build_nc(BS=B // NCORES, CC=64, bufs=3, reps=1, amult16=False):
    C = BS // P
    assert C % CC == 0
    nchunks = C // CC
    CN = CC * N
    E = 2 * CN  # slots per d-plane (x in {q,v} fused)
    OXP = E + S  # ox plane length
    f16 = mybir.dt.float16
    f32 = mybir.dt.float32
    Alu = mybir.AluOpType

    nc = bacc.Bacc("TRN2", num_devices=NCORES)
    m_d = nc.dram_tensor("m", [BS, N], f16, kind="ExternalInput")
    q_d = nc.dram_tensor("q", [D, BS, N], f16, kind="ExternalInput")
    v_d = nc.dram_tensor("v", [D, BS, N], f16, kind="ExternalInput")
    qj_d = nc.dram_tensor("qj", [D, BS, N], f16, kind="ExternalOutput")
    vj_d = nc.dram_tensor("vj", [D, BS, N], f16, kind="ExternalOutput")

    mv = m_d.ap().rearrange("(p c) n -> p c n", p=P)
    qv = q_d.ap().rearrange("d (p c) n -> p d c n", p=P)
    vv = v_d.ap().rearrange("d (p c) n -> p d c n", p=P)
    qjv = qj_d.ap().rearrange("d (p c) n -> p d c n", p=P)
    vjv = vj_d.ap().rearrange("d (p c) n -> p d c n", p=P)

    with tile.TileContext(nc) as tc:
        with (
            tc.tile_pool(name="const", bufs=1) as cpool,
            tc.tile_pool(name="work", bufs=bufs) as pool,
        ):
            g = cpool.tile([P, CN], f16)
            nc.vector.memset(g[:, :], 1.0)
            nc.vector.memset(
                g.rearrange("p (c n) -> p c n", n=N)[:, :, 0:1], 0.0
            )
            # persistent rotating a tiles; slot n=0 pre-zeroed (a_0 = 0)
            ats = []
            for i in range(3):
                at = cpool.tile([P, CN], f16, name=f"a{i}")
                nc.vector.memset(
                    at.rearrange("p (c n) -> p c n", n=N)[:, :, 0:1], 0.0
                )
                ats.append(at)

            for r in range(reps):
                for k in range(nchunks):
                    sl = slice(k * CC, (k + 1) * CC)

                    mt = pool.tile([P, CN], f16, tag="mt")
                    nc.sync.dma_start(
                        out=mt.rearrange("p (c n) -> p c n", n=N),
                        in_=mv[:, sl],
                    )
                    Mt = pool.tile([P, CN], f32, tag="Mt")
                    nc.vector.tensor_tensor_scan(
                        Mt[:, :], g[:, :], mt[:, :], 0.0, Alu.mult, Alu.add
                    )
                    rM = pool.tile([P, CN], f32, tag="rM")
                    nc.vector.reciprocal_approx_fast(rM[:, :], Mt[:, :])

                    a = ats[k % len(ats)]
                    a3 = a.rearrange("p (c n) -> p c n", n=N)
                    if amult16:
                        Mt16 = pool.tile([P, CN], f16, tag="Mt16")
                        nc.scalar.copy(Mt16[:, :], Mt[:, :])
                        rM16 = pool.tile([P, CN], f16, tag="rM16")
                        nc.scalar.copy(rM16[:, :], rM[:, :])
                        Mt163 = Mt16.rearrange("p (c n) -> p c n", n=N)
                        rM163 = rM16.rearrange("p (c n) -> p c n", n=N)
                        nc.vector.tensor_mul(
                            a3[:, :, 1:],
                            Mt163[:, :, 0 : N - 1],
                            rM163[:, :, 1:],
                        )
                    else:
                        Mt3 = Mt.rearrange("p (c n) -> p c n", n=N)
                        rM3 = rM.rearrange("p (c n) -> p c n", n=N)
                        nc.vector.tensor_mul(
                            a3[:, :, 1:], Mt3[:, :, 0 : N - 1], rM3[:, :, 1:]
                        )
                    a2 = pool.tile([P, E], f16, tag="a2")
                    nc.scalar.copy(
                        a2.rearrange("p (x cn) -> p x cn", x=2),
                        a[:, :].unsqueeze(1).broadcast_to([P, 2, CN]),
                    )

                    # fused tile, layout (d, x, c, n), +N pad elements so
                    # the full-width diff and X1 staging stay in bounds
                    xt = pool.tile([P, D * E + N], f16, tag="xt")
                    xt5 = xt[:, 0 : D * E].rearrange(
                        "p (d x c n) -> p d x c n", d=D, x=2, n=N
                    )
                    nc.sync.dma_start(out=xt5[:, :, 0], in_=qv[:, :, sl])
                    nc.sync.dma_start(out=xt5[:, :, 1], in_=vv[:, :, sl])
                    nc.scalar.memzero(xt[:, D * E : D * E + N])

                    # X1[e] = xt[e*N + N] (next row's first elem; any value
                    # works algebraically for the row-0 fixup — it cancels)
                    ne = D * 2 * CC
                    X1 = pool.tile([P, ne], f16, tag="X1")
                    nc.scalar.copy(
                        X1.rearrange("p (e o) -> p e o", o=1),
                        xt[:, N : D * E + N].rearrange(
                            "p (e n) -> p e n", n=N
                        )[:, :, 0:1],
                    )

                    # full-width contiguous diff (2x, aligned)
                    dx = pool.tile([P, D * E], f16, tag="dx")
                    nc.vector.tensor_sub(
                        dx[:, :], xt[:, 1 : D * E + 1], xt[:, 0 : D * E]
                    )

                    # per-d contiguous scans, output shifted by S slots
                    ox = pool.tile([P, OXP * D], f16)
                    for d in range(D):
                        nc.vector.tensor_tensor_scan(
                            ox[:, d * OXP + S : d * OXP + S + E],
                            a2[:, :],
                            dx[:, d * E : (d + 1) * E],
                            0.0,
                            Alu.mult,
                            Alu.add,
                        )
                    # overflow of row e sits at store-slot 0 of row e+1 =
                    # plane position (e+1)*N + S-1; last row's lands in the
                    # pad tail... (E-1)+S ... position E+S-1 < OXP.  Fixup:
                    # out[e,0] = c15[e] = X1[e] - overflow[e].
                    oxp = ox.rearrange("p (d s) -> p d s", d=D)
                    s_view = oxp[:, :, N + S - 1 :: N][:, :, 0 : 2 * CC]
                    r0 = pool.tile([P, ne], f16)
                    r03 = r0.rearrange("p (d e) -> p d e", d=D)
                    nc.gpsimd.tensor_sub(
                        r03, X1.rearrange("p (d e) -> p d e", d=D), s_view
                    )
                    nc.scalar.copy(oxp[:, :, S - 1 :: N][:, :, 0 : 2 * CC], r03)

                    # store block of plane d = positions [S-1, S-1+E)
                    qj_src = oxp[:, :, S - 1 : S - 1 + CN].rearrange(
                        "p d (c n) -> p d c n", n=N
                    )
                    vj_src = oxp[:, :, S - 1 + CN : S - 1 + E].rearrange(
                        "p d (c n) -> p d c n", n=N
                    )
                    # stores issue from the ACT queue to split DMA-issue
                    # load across engines (SP handles the three loads)
                    nc.scalar.dma_start(out=qjv[:, :, sl], in_=qj_src)
                    nc.scalar.dma_start(out=vjv[:, :, sl], in_=vj_src)

    nc.compile()
    return nc


_CACHE = {}


def _get_nc():
    if "nc" not in _CACHE:
        _CACHE["nc"] = build_nc()
    return _CACHE["nc"]


def prep_core_inputs(m, q, v, i):
    BS = B // NCORES
    sl = slice(i * BS, (i + 1) * BS)
    return {
        "m": np.ascontiguousarray(m[sl]).astype(np.float16),
        "q": np.ascontiguousarray(
            np.asarray(q[sl]).astype(np.float16).transpose(2, 0, 1)
        ),
        "v": np.ascontiguousarray(
            np.asarray(v[sl]).astype(np.float16).transpose(2, 0, 1)
        ),
    }


def kernel(m, q, v):
    import os

    os.environ["BASS_NEVER_TRACE"] = "1"
    nc = _get_nc()
    m = np.asarray(m)
    q = np.asarray(q)
    v = np.asarray(v)
    in_maps = [prep_core_inputs(m, q, v, i) for i in range(NCORES)]
    res = run_bass_kernel_spmd(nc, in_maps, list(range(NCORES))).results
    qj = np.concatenate(
        [res[i]["qj"].transpose(1, 2, 0) for i in range(NCORES)], axis=0
    ).astype(np.float32)
    vj = np.concatenate(
        [res[i]["vj"].transpose(1, 2, 0) for i in range(NCORES)], axis=0
    ).astype(np.float32)
    return qj, vj


if __name__ == "__main__":
    import time
    import jax

    with jax.default_device(jax.devices("cpu")[0]):
        import reference

        inputs = {k: np.asarray(x) for k, x in reference.setup_inputs().items()}
        qj_ref, vj_ref = reference.reference(**inputs)
        qj_ref = np.asarray(qj_ref)
        vj_ref = np.asarray(vj_ref)
    t0 = time.time()
    qj, vj = kernel(**inputs)
    print(f"kernel done in {time.time() - t0:.1f}s")
    for name, got, want in (("qj", qj, qj_ref), ("vj", vj, vj_ref)):
        err = np.abs(got - want).max() / np.abs(want).max()
        print(f"{name}: scale-rel absmax err = {err:.3e}")
